# revision 1
# baseline (speedup 1.0000x reference)
"""Trainium2 Bass kernel for nn_DepthwiseXCorr (SiamRPN-style depthwise-xcorr head).

Pipeline per sample (B=128 sharded 16/core across 8 cores, pure data parallel):
  k = relu(bn1(conv3x3(kernel_in, w_ck)))      [256, 5, 5]
  s = relu(bn2(conv3x3(search_in, w_cs)))      [256, 29, 29]
  feat = depthwise_xcorr(s, k)                 [256, 25, 25]
  h = relu(bn3(conv1x1(feat, w_h1)))           [256, 25, 25]
  out = conv1x1(h, w_h2) + b_h2                [10, 25, 25]

Implementation notes:
  - BN scale is folded into conv weights host-side; BN shift + ReLU applied by
    the ACT engine on the PSUM->SBUF eviction (activation = relu(x*1 + bias)).
  - Convs are implicit GEMM on TensorE: input channels (128/chunk) on
    partitions, conv taps accumulate in PSUM, dtype float32r (full PE rate at
    moving free dim >= 256).
  - Depthwise xcorr is a per-channel 5x5 correlation: 25 fused multiply-add
    (scalar_tensor_tensor) ops with per-partition scalars, split DVE/GpSimd.
"""

import numpy as np

EPS = 1e-5
N_CORES = 8
B = 128
B_PER = B // N_CORES  # 16
CIN = 256
H = 256
COUT = 10

_NC_CACHE = {}


def _build_nc(b_per=B_PER, pe_mod=4, gp_pat=(0, 3, 5, 8, 10)):
    """Build the Bass program for one core processing `b_per` samples.

    xcorr routing: every (sample, channel-chunk) unit with index % pe_mod == 0
    runs on the PE via diagonal-weight matmuls; remaining units are split into
    row-halves distributed DVE/GpSimd (halves whose running index mod 12 is in
    gp_pat go to GpSimd).
    """
    import concourse.bacc as bacc
    import concourse.mybir as mybir
    import concourse.tile as tile

    dt = mybir.dt
    f32 = dt.float32
    f32r = dt.float32r
    AF = mybir.ActivationFunctionType
    ALU = mybir.AluOpType

    nc = bacc.Bacc("TRN2", target_bir_lowering=False, debug=False)

    # ---- DRAM tensors (shapes match SBUF tiles exactly; host pre-transposes) ----
    search_d = nc.dram_tensor("search", [b_per, 256, 31, 31], f32r, kind="ExternalInput")
    kin_d = nc.dram_tensor("kin", [2, 128, 9, b_per, 25], f32r, kind="ExternalInput")
    wk_d = nc.dram_tensor("wk", [2, 128, 18, 128], f32r, kind="ExternalInput")
    ws_d = nc.dram_tensor("ws", [2, 128, 18, 128], f32r, kind="ExternalInput")
    w1_d = nc.dram_tensor("w1", [2, 128, 2, 128], f32r, kind="ExternalInput")
    w2_d = nc.dram_tensor("w2", [2, 128, 10], f32r, kind="ExternalInput")
    eye_d = nc.dram_tensor("eye", [128, 128], f32, kind="ExternalInput")
    b1_d = nc.dram_tensor("b1s", [128, 2], f32, kind="ExternalInput")
    b2_d = nc.dram_tensor("b2s", [128, 2], f32, kind="ExternalInput")
    b3_d = nc.dram_tensor("b3s", [128, 2], f32, kind="ExternalInput")
    bh_d = nc.dram_tensor("bhs", [10, 1], f32, kind="ExternalInput")
    y_d = nc.dram_tensor("y", [b_per, 10, 25, 25], f32, kind="ExternalOutput")

    TAPS3 = [(dy, dx) for dy in range(3) for dx in range(3)]
    TAPS5 = [(dy, dx) for dy in range(5) for dx in range(5)]
    # conv_search output row tiling: 29 rows -> two PSUM tiles (N = 435 / 406)
    CS_ROWS = [(0, 15), (15, 14)]
    # h1/h2 output position tiling: 25 rows -> two PSUM tiles (N = 325 / 300)
    H_ROWS = [(0, 13), (13, 12)]

    with tile.TileContext(nc) as tc:
        with (
            tc.tile_pool(name="wpool", bufs=1) as wpool,
            tc.tile_pool(name="kpool", bufs=1) as kpool,
            tc.tile_pool(name="spool", bufs=6) as spool,
            tc.tile_pool(name="fpool", bufs=16) as fpool,
            tc.tile_pool(name="hpool", bufs=6) as hpool,
            tc.tile_pool(name="sfpool", bufs=8) as sfpool,
            tc.tile_pool(name="opool", bufs=3) as opool,
            tc.tile_pool(name="dpool", bufs=3) as dpool,
            tc.tile_pool(name="ps_cs", bufs=3, space="PSUM") as ps_cs,
            tc.tile_pool(name="ps_h", bufs=5, space="PSUM") as ps_h,
        ):
            # ---- conv_kernel inputs first (unblocks phase K quickly) ----
            kin_sb = []
            for c in range(2):
                kt = kpool.tile([128, 9, b_per, 25], f32r, tag=f"kin{c}")
                (nc.scalar if c == 0 else nc.sync).dma_start(kt[:], kin_d[c])
                kin_sb.append(kt)

            # ---- load weights / biases (persistent) ----
            wk_sb = []
            ws_sb = []
            w1_sb = []
            w2_sb = []
            for c in range(2):
                wkt = wpool.tile([128, 18, 128], f32r, tag=f"wk{c}")
                nc.sync.dma_start(wkt[:], wk_d[c])
                wk_sb.append(wkt)
                wst = wpool.tile([128, 18, 128], f32r, tag=f"ws{c}")
                nc.scalar.dma_start(wst[:], ws_d[c])
                ws_sb.append(wst)
                w1t = wpool.tile([128, 2, 128], f32r, tag=f"w1{c}")
                nc.sync.dma_start(w1t[:], w1_d[c])
                w1_sb.append(w1t)
                w2t = wpool.tile([128, 10], f32r, tag=f"w2{c}")
                nc.scalar.dma_start(w2t[:], w2_d[c])
                w2_sb.append(w2t)
            b1_sb = wpool.tile([128, 2], f32, tag="b1")
            nc.sync.dma_start(b1_sb[:], b1_d[:])
            b2_sb = wpool.tile([128, 2], f32, tag="b2")
            nc.sync.dma_start(b2_sb[:], b2_d[:])
            b3_sb = wpool.tile([128, 2], f32, tag="b3")
            nc.sync.dma_start(b3_sb[:], b3_d[:])
            bh_sb = wpool.tile([10, 1], f32, tag="bh")
            nc.sync.dma_start(bh_sb[:], bh_d[:])
            eye_sb = wpool.tile([128, 128], f32, tag="eye")
            nc.scalar.dma_start(eye_sb[:], eye_d[:])

            # ---- phase K: conv_kernel for all samples batched (N = b_per*25) ----
            kf_sb = []
            for cc in range(2):  # output-channel chunk
                psk = ps_cs.tile([128, b_per, 25], f32, tag="ps")
                n_acc = len(TAPS3) * 2
                i = 0
                for (dy, dx) in TAPS3:
                    for ci in range(2):
                        t2c = (dy * 3 + dx) * 2 + cc
                        nc.tensor.matmul(
                            psk[:],
                            wk_sb[ci][:, t2c, :],
                            kin_sb[ci][:, dy * 3 + dx, :, :],
                            start=(i == 0),
                            stop=(i == n_acc - 1),
                        )
                        i += 1
                kf = kpool.tile([128, b_per, 25], f32, tag=f"kf{cc}")
                nc.scalar.activation(kf[:], psk[:], AF.Relu, bias=b1_sb[:, cc : cc + 1])
                kf_sb.append(kf)

            # ---- per-sample pipeline ----
            PE_UNITS = tuple(u for u in range(2 * b_per)
                             if u >= 4 and u % 2 == 0)
            diag_tiles = {}

            def build_diag(u):
                bb, cc = u // 2, u % 2
                dg = dpool.tile([128, 25, 128], f32r, tag="diag")
                # dg[c, t, j] = eye[c, j] * k[c, t]: writes the full tile
                # (zeros off-diagonal), so slot rotation is safe
                mask = eye_sb[:].unsqueeze(1).broadcast_to([128, 25, 128])
                data = kf_sb[cc][:, bb].unsqueeze(2).broadcast_to([128, 25, 128])
                nc.gpsimd.tensor_tensor(dg[:], mask, data, ALU.mult)
                diag_tiles[u] = dg

            unit_idx = 0
            pe_cnt = 0
            half_cnt = 0
            for b in range(b_per):
                # build diagonal weights one sample ahead of their PE unit
                for u in PE_UNITS:
                    if u // 2 == b + 1:
                        build_diag(u)
                # load search input (2 channel chunks)
                sin = []
                for ci in range(2):
                    st = spool.tile([128, 31, 32], f32r, tag="sin")
                    q = (nc.sync, nc.scalar)[(b * 2 + ci) % 2]
                    q.dma_start(st[:, :, 0:31], search_d[b, ci * 128 : (ci + 1) * 128])
                    sin.append(st)

                # conv_search + bn2 + relu -> s_feat [2][128, 29, 29]
                sf = []
                for cc in range(2):
                    sft = sfpool.tile([128, 29, 30], f32r, tag="sf")
                    for (r0, nr) in CS_ROWS:
                        pscs = ps_cs.tile([128, 15, 30], f32, tag="ps")
                        n_acc = len(TAPS3) * 2
                        i = 0
                        for (dy, dx) in TAPS3:
                            t2c = (dy * 3 + dx) * 2 + cc
                            for ci in range(2):
                                nc.tensor.matmul(
                                    pscs[:, :nr, :],
                                    ws_sb[ci][:, t2c, :],
                                    sin[ci][:, dy + r0 : dy + r0 + nr, dx : dx + 30],
                                    start=(i == 0),
                                    stop=(i == n_acc - 1),
                                )
                                i += 1
                        nc.scalar.activation(
                            sft[:, r0 : r0 + nr, 0:29],
                            pscs[:, :nr, 0:29],
                            AF.Relu,
                            bias=b2_sb[:, cc : cc + 1],
                        )
                    sf.append(sft)

                # depthwise xcorr -> feat [cc][half][128, <=13, 25]
                feat = [[None, None], [None, None]]
                for cc in range(2):
                    on_pe = unit_idx in PE_UNITS
                    if on_pe:
                        pe_cnt += 1
                        if unit_idx not in diag_tiles:
                            build_diag(unit_idx)
                        dg = diag_tiles.pop(unit_idx)
                        for hi, (r0, nr) in enumerate(H_ROWS):
                            psx = ps_h.tile([128, 13, 26], f32, tag="ph")
                            for ti, (dy, dx) in enumerate(TAPS5):
                                nc.tensor.matmul(
                                    psx[:, :nr, :],
                                    dg[:, ti, :],
                                    sf[cc][:, dy + r0 : dy + r0 + nr, dx : dx + 26],
                                    start=(ti == 0),
                                    stop=(ti == 24),
                                )
                            ft = fpool.tile([128, 13, 26], f32r, tag="feat")
                            nc.scalar.activation(
                                ft[:, :nr, 0:25], psx[:, :nr, 0:25], AF.Copy
                            )
                            feat[cc][hi] = ft
                    else:
                        for hi, (r0, nr) in enumerate(H_ROWS):
                            eng = nc.vector
                            half_cnt += 1
                            ft = fpool.tile([128, 13, 26], f32r, tag="feat")
                            for ti, (dy, dx) in enumerate(TAPS5):
                                kap = kf_sb[cc][:, b, dy * 5 + dx : dy * 5 + dx + 1]
                                win = sf[cc][:, dy + r0 : dy + r0 + nr, dx : dx + 25]
                                if ti == 0:
                                    eng.tensor_scalar(
                                        ft[:, :nr, 0:25], win, kap, None, ALU.mult
                                    )
                                else:
                                    eng.scalar_tensor_tensor(
                                        ft[:, :nr, 0:25], win, kap, ft[:, :nr, 0:25],
                                        ALU.mult, ALU.add,
                                    )
                            feat[cc][hi] = ft
                    unit_idx += 1

                # h1: 1x1 conv + bn3 + relu -> h1o [2][128, 25, 25]
                h1o = []
                for cc2 in range(2):
                    ht = hpool.tile([128, 25, 26], f32r, tag="h1o")
                    for hi, (r0, nr) in enumerate(H_ROWS):
                        psh = ps_h.tile([128, 13, 26], f32, tag="ph")
                        for ci in range(2):
                            nc.tensor.matmul(
                                psh[:, :nr, :],
                                w1_sb[ci][:, cc2, :],
                                feat[ci][hi][:, :nr, :],
                                start=(ci == 0),
                                stop=(ci == 1),
                            )
                        nc.scalar.activation(
                            ht[:, r0 : r0 + nr, 0:25],
                            psh[:, :nr, 0:25],
                            AF.Relu,
                            bias=b3_sb[:, cc2 : cc2 + 1],
                        )
                    h1o.append(ht)

                # h2: 1x1 conv (+bias) -> out [10, 25, 25]
                osb = opool.tile([10, 25, 25], f32, tag="osb")
                for (r0, nr) in H_ROWS:
                    psh2 = ps_h.tile([10, 13, 26], f32, tag="ph")
                    for ci in range(2):
                        nc.tensor.matmul(
                            psh2[:, :nr, :],
                            w2_sb[ci][:, :],
                            h1o[ci][:, r0 : r0 + nr, :],
                            start=(ci == 0),
                            stop=(ci == 1),
                        )
                    nc.scalar.activation(
                        osb[:, r0 : r0 + nr, :],
                        psh2[:, :nr, 0:25],
                        AF.Identity,
                        bias=bh_sb[:, :],
                    )
                nc.sync.dma_start(y_d[b], osb[:])

    nc.compile()
    return nc


def _get_nc(b_per=B_PER):
    key = b_per
    if key not in _NC_CACHE:
        _NC_CACHE[key] = _build_nc(b_per)
    return _NC_CACHE[key]


def _host_prep(inputs):
    """Fold BN into weights, transpose to lhsT layouts, slice per core."""
    f = np.float32
    kernel = np.ascontiguousarray(inputs["kernel"], dtype=f)
    search = np.ascontiguousarray(inputs["search"], dtype=f)

    def bn_fold(g, b_, m, v):
        scale = g / np.sqrt(v + EPS)
        shift = b_ - m * scale
        return scale.astype(f), shift.astype(f)

    s1, sh1 = bn_fold(inputs["g1"], inputs["b1"], inputs["m1"], inputs["v1"])
    s2, sh2 = bn_fold(inputs["g2"], inputs["b2"], inputs["m2"], inputs["v2"])
    s3, sh3 = bn_fold(inputs["g3"], inputs["b3"], inputs["m3"], inputs["v3"])

    def conv3_lhsT(w, scale):
        # w [co=256, ci=256, 3, 3] * scale[co] -> [cic, ci128, tap*2+coc, co128]
        wf = (w * scale[:, None, None, None]).astype(f)
        wf = wf.reshape(2, 128, 2, 128, 3, 3)  # [coc, co, cic, ci, dy, dx]
        wf = wf.transpose(2, 3, 4, 5, 0, 1)  # [cic, ci, dy, dx, coc, co]
        return np.ascontiguousarray(wf.reshape(2, 128, 18, 128))

    wk = conv3_lhsT(inputs["w_ck"], s1)
    ws = conv3_lhsT(inputs["w_cs"], s2)

    w1 = (inputs["w_h1"][:, :, 0, 0] * s3[:, None]).astype(f)  # [co 256, ci 256]
    w1 = w1.reshape(2, 128, 2, 128).transpose(2, 3, 0, 1)  # [cic, ci, coc, co]
    w1 = np.ascontiguousarray(w1)
    w2 = inputs["w_h2"][:, :, 0, 0].astype(f)  # [10, 256]
    w2 = np.ascontiguousarray(w2.reshape(10, 2, 128).transpose(1, 2, 0))  # [cic, ci, 10]

    weights = dict(
        wk=wk,
        ws=ws,
        w1=w1,
        w2=w2,
        eye=np.eye(128, dtype=f),
        b1s=np.ascontiguousarray(sh1.reshape(2, 128).T),
        b2s=np.ascontiguousarray(sh2.reshape(2, 128).T),
        b3s=np.ascontiguousarray(sh3.reshape(2, 128).T),
        bhs=np.ascontiguousarray(inputs["b_h2"].astype(f).reshape(10, 1)),
    )

    in_maps = []
    for c in range(N_CORES):
        sl = slice(c * B_PER, (c + 1) * B_PER)
        win = np.lib.stride_tricks.sliding_window_view(kernel[sl], (5, 5), axis=(2, 3))
        # win[b, c, dy, dx, y, x] = kernel[b, c, y+dy, x+dx]
        kin = win.reshape(B_PER, 2, 128, 9, 25).transpose(1, 2, 3, 0, 4)
        m = dict(weights)
        m["search"] = search[sl]
        m["kin"] = np.ascontiguousarray(kin)
        in_maps.append(m)
    return in_maps


def run(trace=False, **inputs):
    from concourse import bass_utils

    in_maps = _host_prep(inputs)
    nc = _get_nc()
    try:
        res = bass_utils.run_bass_kernel_spmd(
            nc, in_maps, core_ids=list(range(N_CORES)), trace=trace
        )
    except ModuleNotFoundError:
        # NTFF profiling hook unavailable in this container
        res = bass_utils.run_bass_kernel_spmd(
            nc, in_maps, core_ids=list(range(N_CORES)), trace=False
        )
    y = np.concatenate([res.results[c]["y"] for c in range(N_CORES)], axis=0)
    return y.reshape(B, 10, 25, 25), res


def kernel(**inputs):
    y, _ = run(trace=False, **inputs)
    return y

